# revision 1
# baseline (speedup 1.0000x reference)
"""3x3 MedianBlur (zero-padded) for (8, 3, 1024, 1024) fp32 on 8 trn2 NeuronCores.

Strategy:
  - Pure data parallel: batch element i -> core i (12 MB/core).
  - Per core: 8 row-bands of 128 rows, each processed full-width with all 3
    channels batched into single DVE ops ([128, 3, ~1026] APs) to amortize
    per-instruction init cost.
  - Vertical window alignment comes free from DMA: each band is loaded 3x
    from HBM at row offsets -1/0/+1 (xm/x0/xp, spread over the sync/scalar/
    gpsimd DMA queues so the loads overlap), so the vertical triple for a
    row sits at the same SBUF partition across the three tiles.
  - Exact separable median-of-9 (18 fp32 min/max tensor_tensor ops per
    band, all on the vector engine -- the only 2-tensor elementwise engine
    this toolchain permits):
      stage V: lo/me/hi of the vertical triple          (6 ops)
      stage H: med9 = med3(max3(lo), med3(me), min3(hi)) (12 ops, free-dim
               shifted APs)
    Aggressive in-place tile aliasing (L->u, Hh->v, M->xm, a->x0, cc->xp,
    m1->q) keeps the working set at 6 tags so full-width ops fit SBUF.
  - Zero padding: halo-row tiles memset on edge bands (reordered mid-stream
    to keep them off the critical path); 1-col zero borders on the x tiles
    make the sorted triple of a padded column (0,0,0).
  - Final op + store split by channel so the last store overlaps compute.
  - Simulated (cost-model) makespan: ~483 us/core; DVE wall-to-wall
    (470 us busy, zero gaps) = the stream floor for this 18-op network.

The walrus build here accepts at most 1 inline sync wait per instruction
(2 on EventSemaphore); Tile emits more, so _legalize_waits() spills excess
waits onto same-engine NoOps placed immediately before the instruction.
"""
import sys

sys.path.insert(0, "/opt/trn_rl_repo")

import numpy as np

import concourse.bass as bass
import concourse.mybir as mybir
from concourse.bass_utils import run_bass_kernel_spmd
from concourse.tile import TileContext


C, H, W = 3, 1024, 1024
P = 128
NT = H // P
S = 1          # W strips
SW = W // S    # outputs per strip
SP = SW + 2    # padded strip width
F32 = mybir.dt.float32
MIN = mybir.AluOpType.min
MAX = mybir.AluOpType.max


def _legalize_waits(nc):
    """Split sync_info.on_wait lists that exceed this walrus's per-instruction
    capacity (1; 2 for EventSemaphore) onto preceding same-engine NoOps."""
    for f in nc.m.functions:
        for bb in f.blocks:
            new_insts = []
            for ins in bb.instructions:
                si = ins.sync_info
                cap = 2 if ins.opcode == "EventSemaphore" else 1
                if si is not None and len(si.on_wait) > cap:
                    waits = list(si.on_wait)
                    extra, keep = waits[:-cap], waits[-cap:]
                    for w in extra:
                        nop = mybir.InstNoOp(
                            name=nc.get_next_instruction_name(),
                            ins=[],
                            outs=[],
                            engine=ins.engine,
                        )
                        nop.sync_info = mybir.SyncInfo(on_wait=[w], on_update=[])
                        new_insts.append(nop)
                    ins.sync_info = mybir.SyncInfo(
                        on_wait=keep, on_update=list(si.on_update)
                    )
                new_insts.append(ins)
            bb.instructions = new_insts



def build(bufs=2):
    nc = bass.Bass()
    x = nc.dram_tensor("x", [C, H, W], F32, kind="ExternalInput")
    y = nc.dram_tensor("y", [C, H, W], F32, kind="ExternalOutput")
    tt = nc.vector.tensor_tensor

    with TileContext(nc) as tc:
        with (
            tc.tile_pool(name="deep", bufs=bufs + 1) as dpool,
            tc.tile_pool(name="shallow", bufs=bufs) as pool,
        ):
            # edge bands (t=0, t=NT-1) mid-stream so their halo memsets
            # overlap compute instead of delaying the first loads
            order = [1, 2, 0, 3, 4, NT - 1, 5, 6]
            for ui, t in enumerate(order):
                r0 = t * P
                for s in range(S):
                    # tile cols 0..SP-1 <-> x cols [512s-1 .. 512s+512]
                    cl = s * SW - 1          # leftmost x col (may be -1)
                    x0 = dpool.tile([P, C, SP], F32, tag="x0")
                    xm = dpool.tile([P, C, SP], F32, tag="xm")
                    xp = dpool.tile([P, C, SP], F32, tag="xp")
                    # clipped col range present in DRAM
                    dl = max(cl, 0)
                    dr = min(cl + SP, W)     # exclusive
                    o0 = dl - cl             # tile col where DMA data starts
                    n = dr - dl
                    if t == 0:
                        nc.gpsimd.memset(xm[:], 0.0)
                    if t == NT - 1:
                        nc.gpsimd.memset(xp[:], 0.0)
                    for z in (x0, xm, xp):
                        if o0 > 0:
                            nc.gpsimd.memset(z[:, :, 0:1], 0.0)
                        if o0 + n < SP:
                            nc.gpsimd.memset(z[:, :, SP - 1 : SP], 0.0)
                    # one DMA per tensor: iteration order (row, channel, col).
                    # First processed unit: per-channel loads + per-channel
                    # stage V, so DVE starts after 1/3 of the load data.
                    chunks = [(c, c + 1) for c in range(C)] if ui == 0 else [(0, C)]
                    for c0, c1 in chunks:
                        nc.sync.dma_start(
                            x0[:, c0:c1, o0 : o0 + n],
                            x[c0:c1, r0 : r0 + P, dl:dr].rearrange("c r w -> r c w"),
                        )
                        if t == 0:
                            nc.scalar.dma_start(
                                xm[1:P, c0:c1, o0 : o0 + n],
                                x[c0:c1, 0 : P - 1, dl:dr].rearrange("c r w -> r c w"),
                            )
                        else:
                            nc.scalar.dma_start(
                                xm[:, c0:c1, o0 : o0 + n],
                                x[c0:c1, r0 - 1 : r0 + P - 1, dl:dr].rearrange(
                                    "c r w -> r c w"
                                ),
                            )
                        if t == NT - 1:
                            nc.gpsimd.dma_start(
                                xp[0 : P - 1, c0:c1, o0 : o0 + n],
                                x[c0:c1, r0 + 1 : r0 + P, dl:dr].rearrange(
                                    "c r w -> r c w"
                                ),
                            )
                        else:
                            nc.gpsimd.dma_start(
                                xp[:, c0:c1, o0 : o0 + n],
                                x[c0:c1, r0 + 1 : r0 + P + 1, dl:dr].rearrange(
                                    "c r w -> r c w"
                                ),
                            )

                    # stage V with full in-place reuse (tile -> final contents):
                    #   u -> L, v -> Hh, xm -> M, x0 -> t2 (scratch)
                    u = pool.tile([P, C, SP], F32, tag="u")
                    v = pool.tile([P, C, SP], F32, tag="v")
                    for c0, c1 in chunks:
                        tt(u[:, c0:c1], xm[:, c0:c1], x0[:, c0:c1], MIN)
                        tt(v[:, c0:c1], xm[:, c0:c1], x0[:, c0:c1], MAX)
                        tt(x0[:, c0:c1], v[:, c0:c1], xp[:, c0:c1], MIN)  # t2
                        tt(xm[:, c0:c1], u[:, c0:c1], x0[:, c0:c1], MAX)  # M
                        tt(u[:, c0:c1], u[:, c0:c1], xp[:, c0:c1], MIN)  # L
                        tt(v[:, c0:c1], v[:, c0:c1], xp[:, c0:c1], MAX)  # Hh

                    # stage H, reusing dead tiles: a->x0, cc->xp, m1->q
                    q = pool.tile([P, C, SP - 1], F32, tag="q")
                    tt(x0[:, :, 0 : SP - 1], u[:, :, 0 : SP - 1], u[:, :, 1:SP], MAX)  # a
                    tt(x0[:, :, 0:SW], x0[:, :, 0:SW], u[:, :, 2:SP], MAX)  # A
                    tt(xp[:, :, 0 : SP - 1], v[:, :, 0 : SP - 1], v[:, :, 1:SP], MIN)  # c
                    tt(xp[:, :, 0:SW], xp[:, :, 0:SW], v[:, :, 2:SP], MIN)  # Cc
                    tt(q[:], xm[:, :, 0 : SP - 1], xm[:, :, 1:SP], MAX)
                    tt(q[:, :, 0:SW], q[:, :, 0:SW], xm[:, :, 2:SP], MIN)  # b1
                    tt(xm[:, :, 0 : SP - 1], xm[:, :, 0 : SP - 1], xm[:, :, 1:SP], MIN)  # p
                    tt(xm[:, :, 0:SW], xm[:, :, 0:SW], q[:, :, 0:SW], MAX)  # B
                    tt(q[:, :, 0:SW], x0[:, :, 0:SW], xm[:, :, 0:SW], MIN)  # m1
                    tt(x0[:, :, 0:SW], x0[:, :, 0:SW], xm[:, :, 0:SW], MAX)  # m2
                    tt(xp[:, :, 0:SW], x0[:, :, 0:SW], xp[:, :, 0:SW], MIN)  # m3
                    # final op + store split by channel so the store of the
                    # first chunk overlaps compute of the second
                    tt(q[:, 0:2, 0:SW], q[:, 0:2, 0:SW], xp[:, 0:2, 0:SW], MAX)
                    nc.scalar.dma_start(
                        y[0:2, r0 : r0 + P, s * SW : (s + 1) * SW].rearrange(
                            "c r w -> r c w"
                        ),
                        q[:, 0:2, 0:SW],
                    )
                    tt(q[:, 2:3, 0:SW], q[:, 2:3, 0:SW], xp[:, 2:3, 0:SW], MAX)
                    nc.scalar.dma_start(
                        y[2:3, r0 : r0 + P, s * SW : (s + 1) * SW].rearrange(
                            "c r w -> r c w"
                        ),
                        q[:, 2:3, 0:SW],
                    )

    _legalize_waits(nc)
    return nc


_NC = None


def kernel(input):
    global _NC
    if _NC is None:
        _NC = build()
    input = np.asarray(input, dtype=np.float32)
    in_maps = [{"x": np.ascontiguousarray(input[i])} for i in range(input.shape[0])]
    res = run_bass_kernel_spmd(_NC, in_maps, core_ids=list(range(len(in_maps))))
    return np.stack([r["y"] for r in res.results], axis=0)



# revision 2
# speedup vs baseline: 1.9151x; 1.9151x over previous
"""3x3 MedianBlur (zero-padded) for (8, 3, 1024, 1024) fp32 on 8 trn2 NeuronCores.

v2 strategy (vs 480us baseline):
  - Pure data parallel: batch element i -> core i.
  - bf16 compute: tolerance is 2e-2 rel; bf16 rounding of the exact median
    network costs <= ~2^-7 rel. Host converts fp32->bf16 before upload and
    bf16->fp32 after download, halving HBM traffic AND putting every DVE
    tensor_tensor in 2x_1p mode (0.5 cyc/elem instead of 1).
  - Flattened row-group layout: partition p holds image rows 8p-1..8p+8
    (8 output rows + 1 halo row each side) as 10 rows x 1026 cols (1 zero
    pad col each side) in the free dim. Both 3x3 window shifts are then
    free-dim offsets (+-1026 vertical, +-1 horizontal): one DMA load per
    channel instead of 3 row-shifted loads, no cross-partition traffic.
  - Exact separable median-of-9: 18 bf16 min/max tensor_tensor ops per
    channel at FD~8208, all on DVE (the only tensor_tensor engine this
    toolchain's walrus accepts; TT/Pool and InstPool both fail ISA checks).
    DVE engine time ~= 54 * (8208*0.521 + 60) ns ~= 234 us; DMA (4.6 MB/chan
    in+out bf16) hides under compute.
  - Pads/halos zeroed by Pool-engine memsets; channel c+1 loads overlap
    channel c compute via bufs=2 on the x/out tiles.

The walrus build accepts at most 1 inline sync wait per instruction
(2 on EventSemaphore); Tile emits more, so _legalize_waits() spills excess
waits onto same-engine NoOps placed immediately before the instruction.
"""
import sys

sys.path.insert(0, "/opt/trn_rl_repo")

import numpy as np

import concourse.bass as bass
import concourse.mybir as mybir
from concourse.bass_utils import run_bass_kernel_spmd
from concourse.tile import TileContext

C, H, W = 3, 1024, 1024
P = 128
RP = H // P          # rows per partition (8)
WP = W + 2           # padded row width (1026)
NR = RP + 2          # rows resident per partition incl. halo (10)
BF16 = mybir.dt.bfloat16
MIN = mybir.AluOpType.min
MAX = mybir.AluOpType.max


def _legalize_waits(nc):
    """Split sync_info.on_wait lists that exceed this walrus's per-instruction
    capacity (1; 2 for EventSemaphore) onto preceding same-engine NoOps."""
    for f in nc.m.functions:
        for bb in f.blocks:
            new_insts = []
            for ins in bb.instructions:
                si = ins.sync_info
                cap = 2 if ins.opcode == "EventSemaphore" else 1
                if si is not None and len(si.on_wait) > cap:
                    waits = list(si.on_wait)
                    extra, keep = waits[:-cap], waits[-cap:]
                    for w in extra:
                        nop = mybir.InstNoOp(
                            name=nc.get_next_instruction_name(),
                            ins=[],
                            outs=[],
                            engine=ins.engine,
                        )
                        nop.sync_info = mybir.SyncInfo(on_wait=[w], on_update=[])
                        new_insts.append(nop)
                    ins.sync_info = mybir.SyncInfo(
                        on_wait=keep, on_update=list(si.on_update)
                    )
                new_insts.append(ins)
            bb.instructions = new_insts


def build():
    nc = bass.Bass()
    xd = nc.dram_tensor("x", [C, H, W], BF16, kind="ExternalInput")
    yd = nc.dram_tensor("y", [C, H, W], BF16, kind="ExternalOutput")
    tt = nc.vector.tensor_tensor

    with TileContext(nc) as tc:
        with (
            tc.tile_pool(name="io", bufs=2) as iop,
            tc.tile_pool(name="work", bufs=1) as wp,
        ):
            for c in range(C):
                # ---- load: x tile holds rows 8p-1 .. 8p+8, each padded to
                # 1026 cols (zero col 0 and 1025). Slot k = image row 8p-1+k.
                x = iop.tile([P, NR, WP], BF16, tag="x")
                # zero pads: left/right cols of every row; halo rows at the
                # image top (partition 0 slot 0) and bottom (partition 127
                # slot 9), which have no source rows.
                nc.gpsimd.memset(x[:, :, 0:1], 0.0)
                nc.gpsimd.memset(x[:, :, WP - 1 : WP], 0.0)
                nc.gpsimd.memset(x[0:1, 0:1, :], 0.0)
                # compute engines must start at partition 0/32/64/96: zero
                # slot 9 for the whole last quadrant; the bottom-halo DMA
                # then overwrites partitions 96..126, leaving 127 zero.
                nc.gpsimd.memset(x[96:P, NR - 1 : NR, :], 0.0)
                # main: slots 1..8 = rows 8p..8p+7 (always valid)
                nc.sync.dma_start(
                    x[:, 1 : RP + 1, 1 : W + 1],
                    xd[c : c + 1, :, :].rearrange("c (p k) w -> p (c k) w", k=RP),
                )
                # top halo: slot 0 = row 8p-1, partitions 1..127 (rows 7,15,..)
                nc.scalar.dma_start(
                    x[1:P, 0:1, 1 : W + 1],
                    xd[c : c + 1, RP - 1 : H - 1, :].rearrange(
                        "c (p k) w -> p (c k) w", k=RP
                    )[:, 0:1, :],
                )
                # bottom halo: slot 9 = row 8p+8, partitions 0..126
                nc.gpsimd.dma_start(
                    x[0 : P - 1, NR - 1 : NR, 1 : W + 1],
                    xd[c : c + 1, RP:H, :].rearrange(
                        "c (p k) w -> p (c k) w", k=RP
                    )[:, 0:1, :],
                )

                # ---- vertical stage: lo/me/hi of each column triple.
                # xm/x0/xp are the same tile shifted by one row-slot.
                xm = x[:, 0:RP, :]
                x0 = x[:, 1 : RP + 1, :]
                xp = x[:, 2 : RP + 2, :]
                u = wp.tile([P, RP, WP], BF16, tag="u")
                v = wp.tile([P, RP, WP], BF16, tag="v")
                w = wp.tile([P, RP, WP], BF16, tag="w")
                tt(u[:], xm, x0, MIN)
                tt(v[:], xm, x0, MAX)
                tt(w[:], v[:], xp, MIN)
                tt(w[:], u[:], w[:], MAX)  # M  (me)    in-place
                tt(u[:], u[:], xp, MIN)    # L  (lo)    in-place
                tt(v[:], v[:], xp, MAX)    # Hh (hi)    in-place

                # ---- horizontal stage (shift +-1 within each padded row):
                # med9 = med3(max3(L), med3(M), min3(Hh))
                t1 = wp.tile([P, RP, WP - 1], BF16, tag="t1")
                t2 = wp.tile([P, RP, WP - 1], BF16, tag="t2")
                out = iop.tile([P, RP, W], BF16, tag="out")
                W1, W2 = WP - 1, WP - 2  # 1025, 1024
                tt(t1[:], u[:, :, 0:W1], u[:, :, 1:WP], MAX)              # a
                tt(t1[:, :, 0:W2], t1[:, :, 0:W2], u[:, :, 2:WP], MAX)    # A
                tt(u[:, :, 0:W1], v[:, :, 0:W1], v[:, :, 1:WP], MIN)      # cc
                tt(u[:, :, 0:W2], u[:, :, 0:W2], v[:, :, 2:WP], MIN)      # Cc
                tt(t2[:], w[:, :, 0:W1], w[:, :, 1:WP], MAX)              # q
                tt(v[:, :, 0:W1], w[:, :, 0:W1], w[:, :, 1:WP], MIN)      # p
                tt(t2[:, :, 0:W2], t2[:, :, 0:W2], w[:, :, 2:WP], MIN)    # b1
                tt(t2[:, :, 0:W2], v[:, :, 0:W2], t2[:, :, 0:W2], MAX)    # B
                tt(v[:, :, 0:W2], t1[:, :, 0:W2], t2[:, :, 0:W2], MIN)    # m1
                tt(t1[:, :, 0:W2], t1[:, :, 0:W2], t2[:, :, 0:W2], MAX)   # m2
                tt(t1[:, :, 0:W2], t1[:, :, 0:W2], u[:, :, 0:W2], MIN)    # m3
                tt(out[:], v[:, :, 0:W2], t1[:, :, 0:W2], MAX)            # med9

                nc.scalar.dma_start(
                    yd[c : c + 1, :, :].rearrange("c (p k) w -> p (c k) w", k=RP),
                    out[:],
                )

    _legalize_waits(nc)
    return nc


_NC = None


def kernel(input):
    import ml_dtypes

    global _NC
    if _NC is None:
        _NC = build()
    xb = np.asarray(input, dtype=np.float32).astype(ml_dtypes.bfloat16)
    in_maps = [{"x": np.ascontiguousarray(xb[i])} for i in range(xb.shape[0])]
    res = run_bass_kernel_spmd(_NC, in_maps, core_ids=list(range(len(in_maps))))
    return np.stack([r["y"] for r in res.results], axis=0).astype(np.float32)


# revision 3
# speedup vs baseline: 1.9622x; 1.0246x over previous
"""3x3 MedianBlur (zero-padded) for (8, 3, 1024, 1024) fp32 on 8 trn2 NeuronCores.

v3 = v2 (bf16 + flattened row-group layout, all-DVE 18-op exact median
network) + schedule tightening: the first and last channels are processed
in two row-group halves so the first DVE op starts after ~60% of the first
load, and the final store tail is halved. See kernel_v2 docstring for the
core design.
"""
import sys

sys.path.insert(0, "/opt/trn_rl_repo")

import numpy as np

import concourse.bass as bass
import concourse.mybir as mybir
from concourse.bass_utils import run_bass_kernel_spmd
from concourse.tile import TileContext

C, H, W = 3, 1024, 1024
P = 128
RP = H // P          # rows per partition (8)
WP = W + 2           # padded row width (1026)
NR = RP + 2          # rows resident per partition incl. halo (10)
BF16 = mybir.dt.bfloat16
MIN = mybir.AluOpType.min
MAX = mybir.AluOpType.max


def _legalize_waits(nc):
    """Split sync_info.on_wait lists that exceed this walrus's per-instruction
    capacity (1; 2 for EventSemaphore) onto preceding same-engine NoOps."""
    for f in nc.m.functions:
        for bb in f.blocks:
            new_insts = []
            for ins in bb.instructions:
                si = ins.sync_info
                cap = 2 if ins.opcode == "EventSemaphore" else 1
                if si is not None and len(si.on_wait) > cap:
                    waits = list(si.on_wait)
                    extra, keep = waits[:-cap], waits[-cap:]
                    for w in extra:
                        nop = mybir.InstNoOp(
                            name=nc.get_next_instruction_name(),
                            ins=[],
                            outs=[],
                            engine=ins.engine,
                        )
                        nop.sync_info = mybir.SyncInfo(on_wait=[w], on_update=[])
                        new_insts.append(nop)
                    ins.sync_info = mybir.SyncInfo(
                        on_wait=keep, on_update=list(si.on_update)
                    )
                new_insts.append(ins)
            bb.instructions = new_insts


def build(segs_for=None):
    nc = bass.Bass()
    xd = nc.dram_tensor("x", [C, H, W], BF16, kind="ExternalInput")
    yd = nc.dram_tensor("y", [C, H, W], BF16, kind="ExternalOutput")
    tt = nc.vector.tensor_tensor

    if segs_for is None:
        # tuned in cost-model sim: tiny first segment starts DVE as soon as
        # the first load lands; tiny last segment shortens the store tail
        segs_for = {
            0: [(0, 1), (1, 4), (4, RP)],
            1: [(0, RP)],
            2: [(0, 7), (7, RP)],
        }

    with TileContext(nc) as tc:
        with (
            tc.tile_pool(name="io", bufs=2) as iop,
            tc.tile_pool(name="work", bufs=1) as wp,
        ):
            for c in range(C):
                segs = segs_for[c]
                x = iop.tile([P, NR, WP], BF16, tag="x")
                nc.gpsimd.memset(x[:, :, 0:1], 0.0)
                nc.gpsimd.memset(x[:, :, WP - 1 : WP], 0.0)
                nc.gpsimd.memset(x[0:1, 0:1, :], 0.0)
                # compute engines must start at partition 0/32/64/96: zero
                # slot 9 for the whole last quadrant; the bottom-halo DMA
                # then overwrites partitions 96..126, leaving 127 zero.
                nc.gpsimd.memset(x[96:P, NR - 1 : NR, :], 0.0)
                # top halo first (gates the first V op), then main rows in
                # one chunk per segment, then bottom halo.
                nc.scalar.dma_start(
                    x[1:P, 0:1, 1 : W + 1],
                    xd[c : c + 1, RP - 1 : H - 1, :].rearrange(
                        "c (p k) w -> p (c k) w", k=RP
                    )[:, 0:1, :],
                )
                for k0, k1 in segs:
                    nc.sync.dma_start(
                        x[:, k0 + 1 : k1 + 1, 1 : W + 1],
                        xd[c : c + 1, :, :].rearrange(
                            "c (p k) w -> p (c k) w", k=RP
                        )[:, k0:k1, :],
                    )
                nc.gpsimd.dma_start(
                    x[0 : P - 1, NR - 1 : NR, 1 : W + 1],
                    xd[c : c + 1, RP:H, :].rearrange(
                        "c (p k) w -> p (c k) w", k=RP
                    )[:, 0:1, :],
                )

                u = wp.tile([P, RP, WP], BF16, tag="u")
                v = wp.tile([P, RP, WP], BF16, tag="v")
                w = wp.tile([P, RP, WP], BF16, tag="w")
                t1 = wp.tile([P, RP, WP - 1], BF16, tag="t1")
                t2 = wp.tile([P, RP, WP - 1], BF16, tag="t2")
                out = iop.tile([P, RP, W], BF16, tag="out")
                W1, W2 = WP - 1, WP - 2  # 1025, 1024

                for k0, k1 in segs:
                    # vertical stage: lo/me/hi of column triples for output
                    # row-slots [k0:k1) (x slots [k0:k1+2))
                    xm = x[:, k0 : k1, :]
                    x0 = x[:, k0 + 1 : k1 + 1, :]
                    xp = x[:, k0 + 2 : k1 + 2, :]
                    U = u[:, k0:k1]
                    V = v[:, k0:k1]
                    Wt = w[:, k0:k1]
                    tt(U[:, :, :], xm, x0, MIN)
                    tt(V[:, :, :], xm, x0, MAX)
                    tt(Wt[:, :, :], V[:, :, :], xp, MIN)
                    tt(Wt[:, :, :], U[:, :, :], Wt[:, :, :], MAX)  # M (me)
                    tt(U[:, :, :], U[:, :, :], xp, MIN)            # L (lo)
                    tt(V[:, :, :], V[:, :, :], xp, MAX)            # H (hi)

                    # horizontal: med9 = med3(max3(L), med3(M), min3(H))
                    T1 = t1[:, k0:k1]
                    T2 = t2[:, k0:k1]
                    tt(T1[:, :, :], U[:, :, 0:W1], U[:, :, 1:WP], MAX)            # a
                    tt(T1[:, :, 0:W2], T1[:, :, 0:W2], U[:, :, 2:WP], MAX)        # A
                    tt(U[:, :, 0:W1], V[:, :, 0:W1], V[:, :, 1:WP], MIN)          # cc
                    tt(U[:, :, 0:W2], U[:, :, 0:W2], V[:, :, 2:WP], MIN)          # Cc
                    tt(T2[:, :, :], Wt[:, :, 0:W1], Wt[:, :, 1:WP], MAX)          # q
                    tt(V[:, :, 0:W1], Wt[:, :, 0:W1], Wt[:, :, 1:WP], MIN)        # p
                    tt(T2[:, :, 0:W2], T2[:, :, 0:W2], Wt[:, :, 2:WP], MIN)       # b1
                    tt(T2[:, :, 0:W2], V[:, :, 0:W2], T2[:, :, 0:W2], MAX)        # B
                    tt(V[:, :, 0:W2], T1[:, :, 0:W2], T2[:, :, 0:W2], MIN)        # m1
                    tt(T1[:, :, 0:W2], T1[:, :, 0:W2], T2[:, :, 0:W2], MAX)       # m2
                    tt(T1[:, :, 0:W2], T1[:, :, 0:W2], U[:, :, 0:W2], MIN)        # m3
                    tt(out[:, k0:k1, :], V[:, :, 0:W2], T1[:, :, 0:W2], MAX)      # med9

                    nc.scalar.dma_start(
                        yd[c : c + 1, :, :].rearrange(
                            "c (p k) w -> p (c k) w", k=RP
                        )[:, k0:k1, :],
                        out[:, k0:k1, :],
                    )

    _legalize_waits(nc)
    return nc


_NC = None


def kernel(input):
    import ml_dtypes

    global _NC
    if _NC is None:
        _NC = build()
    xb = np.asarray(input, dtype=np.float32).astype(ml_dtypes.bfloat16)
    in_maps = [{"x": np.ascontiguousarray(xb[i])} for i in range(xb.shape[0])]
    res = run_bass_kernel_spmd(_NC, in_maps, core_ids=list(range(len(in_maps))))
    return np.stack([r["y"] for r in res.results], axis=0).astype(np.float32)
